# revision 25
# baseline (speedup 1.0000x reference)
"""Masked cross-attention kernel for Trainium2 (8 NeuronCores, SPMD).

Problem: B=16 batches of softmax(mask(Q@K^T/sqrt(D)))@V with
Lq=Lk=2048, D=DV=256.  The reference zeroes masked scores (NOT -inf)
before the softmax, so masked keys still contribute exp(0)=1 to the
denominator and weight 1/denom on V rows.

Strategy (all host prep is exact):
  * Zero K rows at k >= valid_length[b] on the host.  Then Q @ K^T is
    *exactly* 0.0 at masked positions - identical to the reference's
    jnp.where - and no mask tensor is needed on-device.
  * Pre-transpose Q and K to [D, L] layout on the host so both matmul
    operands stream naturally (contraction on the partition dim).
  * Append a ones-column to V.  P @ [V | 1] then yields the softmax
    denominator as output column 256 for free.
  * bf16 matmul inputs (fp32 PSUM accumulate), fp32 softmax math.
  * All per-batch inputs are packed host-side into ONE blob tensor
    [128 partitions x cols] and loaded in 3 big segment DMAs (a single
    DMA fans out across all 16 SDMA engines; many small DMAs each pay
    a ~2us completion latency and fair-share the engines).

Per core: 2 batches.  Per batch, for each 512-wide q tile:
  stage 1: S^T[k,q] tiles in PSUM (Kt.T @ Qt), exp via ScalarE
           (scale=1/16 folded in) -> P^T bf16 in SBUF
  stage 2: O[q,v] = (P^T).T @ [V|1] accumulated over k chunks in PSUM;
           divide by column 256 (DVE reciprocal + per-partition mul).
Stage 1 of q-tile i+1 is emitted before stage 2 of q-tile i so the PE
never stalls on the ScalarE exp chain.
"""

import numpy as np
import ml_dtypes

import concourse.bass as bass
import concourse.mybir as mybir
import concourse.tile as tile
from concourse import bacc
from concourse.bass_utils import run_bass_kernel_spmd

B, LQ, LK, D, DV = 16, 2048, 2048, 256, 256
N_CORES = 8
BPC = B // N_CORES  # batches per core

QT = 512            # q-tile width (stage-1 moving free dim)
NQT = LQ // QT      # 4
KT = 128            # k-tile (partition dim of S^T)
NKT = LK // KT      # 16
KG = 2              # k-tiles per PSUM/exp group
NKG = NKT // KG     # 8
NDC = D // 128      # contraction chunks (2)
QS = 128            # q-subtile for stage 2
NQS = QT // QS      # 4
VF = DV + 1         # 257: V plus the ones column
WARMUP_MMS = 14     # HAM warm-up matmuls bridging the initial DMA wait

# Blob column layout (per partition, bf16).  Segments sized so the
# latency-critical first working set (kt + qt0) splits evenly across
# the two independent HWDGE rings (sync + scalar), which run FIFO-serial
# per ring at ~170GB/s each:
#   seg A1: kt_c0 2048 | qt0_c0 512   -> 2560   (sync)
#   seg A2: kt_c1 2048 | qt0_c1 512   -> 2560   (scalar)
#   seg B:  qt1 1024   | v1a 8*VF=2056 -> 3080  (sync)
#   seg C:  qt2 1024   | v1b 2056      -> 3080  (scalar)
#   seg D:  qt3 1024                   -> 1024  (sync)
SEG_A = LK + QT                      # 2560 (x2)
SEG_B = NDC * QT + (NKT // 2) * VF   # 3080
SEG_C = SEG_B                        # 3080
SEG_D = NDC * QT                     # 1024
BLOB = 2 * SEG_A + SEG_B + SEG_C + SEG_D  # 12304

_BF16 = mybir.dt.bfloat16
_F32 = mybir.dt.float32

_NC_CACHE = {}


def _build_nc():
    nc = bacc.Bacc("TRN2", target_bir_lowering=False, debug=False,
                   num_devices=N_CORES)

    blob_d = nc.declare_dram_parameter("blob", [BPC, 128, BLOB], _BF16,
                                       isOutput=False)
    out_d = nc.declare_dram_parameter("out", [BPC, LQ, DV], _F32,
                                      isOutput=True)

    with tile.TileContext(nc) as tc:
        with (
            tc.tile_pool(name="seg", bufs=2) as seg_pool,
            tc.tile_pool(name="p", bufs=2) as p_pool,
            tc.tile_pool(name="osb", bufs=4) as o_pool,
            tc.tile_pool(name="small", bufs=8) as small_pool,
            tc.tile_pool(name="ps_s", bufs=2, space="PSUM") as ps_s,
            tc.tile_pool(name="ps_o", bufs=4, space="PSUM") as ps_o,
        ):
            SEG_TBL = [  # (offset, size)
                (0, SEG_A), (SEG_A, SEG_A), (2 * SEG_A, SEG_B),
                (2 * SEG_A + SEG_B, SEG_C), (2 * SEG_A + SEG_B + SEG_C, SEG_D),
            ]

            def load_batch(b):
                segs = []
                # batch 0 is latency-critical: A1/B/D on the sync ring,
                # A2/C on the scalar ring, concurrently.  batch 1 loads
                # all on sync (no deadline; keeps ACT free for exps).
                engs = ([nc.gpsimd, nc.scalar, nc.sync, nc.scalar, nc.sync]
                        if b == 0 else [nc.sync] * 5)
                for si, (lo, n) in enumerate(SEG_TBL):
                    t = seg_pool.tile([128, n], _BF16, tag=f"seg{si}",
                                      name=f"seg{si}_b{b}")
                    engs[si].dma_start(out=t, in_=blob_d[b, :, lo:lo + n])
                    segs.append(t)
                return segs

            def kt_slice(segs, c, kj):
                return segs[c][:, kj * KT:(kj + 1) * KT]

            def qt_slice(segs, qi, c):
                if qi == 0:
                    return segs[c][:, LK:LK + QT]
                offs = [(2, 0), (3, 0), (4, 0)]
                si, o = offs[qi - 1]
                return segs[si][:, o + c * QT:o + (c + 1) * QT]

            def v1_slice(segs, kj):
                if kj < NKT // 2:
                    return segs[2][:, NDC * QT + kj * VF:NDC * QT + (kj + 1) * VF]
                kj -= NKT // 2
                return segs[3][:, NDC * QT + kj * VF:NDC * QT + (kj + 1) * VF]

            def stage1(segs, qi):
                """S^T = Kt.T @ Qt for one 512-wide q tile; exp -> P^T bf16."""
                p_sb = p_pool.tile([128, NKT * QT], _BF16, tag="p")
                for g in range(NKG):
                    ps = ps_s.tile([128, KG * QT], _F32, tag="s")
                    for h in range(KG):
                        kj = g * KG + h
                        for c in range(NDC):
                            nc.tensor.matmul(
                                ps[:, h * QT:(h + 1) * QT],
                                lhsT=kt_slice(segs, c, kj),
                                rhs=qt_slice(segs, qi, c),
                                start=(c == 0),
                                stop=(c == NDC - 1),
                            )
                    nc.scalar.activation(
                        out=p_sb[:, g * KG * QT:(g + 1) * KG * QT], in_=ps,
                        func=mybir.ActivationFunctionType.Exp,
                        scale=1.0 / 16.0)
                return p_sb

            def stage2(segs, b, qi, p_sb):
                """O = P @ [V|1]; normalize by the ones column; DMA out."""
                for s in range(NQS):
                    o_ps = ps_o.tile([128, VF], _F32, tag="o")
                    for kj in range(NKT):
                        nc.tensor.matmul(
                            o_ps,
                            lhsT=p_sb[:, kj * QT + s * QS:kj * QT + (s + 1) * QS],
                            rhs=v1_slice(segs, kj),
                            start=(kj == 0), stop=(kj == NKT - 1),
                        )
                    recip = small_pool.tile([128, 1], _F32, tag="r")
                    nc.vector.reciprocal(out=recip, in_=o_ps[:, DV:DV + 1])
                    o_sb = o_pool.tile([128, DV], _F32, tag="o_sb")
                    nc.vector.tensor_scalar_mul(
                        out=o_sb, in0=o_ps[:, :DV], scalar1=recip)
                    q0 = qi * QT + s * QS
                    nc.sync.dma_start(out=out_d[b, q0:q0 + QS, :], in_=o_sb)

            # HAM warm-up: matmuls on an UNINITIALIZED tile (no producer ->
            # no waits -> PE starts right after its preamble) into a
            # throwaway PSUM group.  The tiny DVE read keeps DCE away.
            warm = small_pool.tile([128, QT], _BF16, tag="warm")
            wps = ps_o.tile([128, QT], _F32, tag="o", name="warm_ps")
            for w in range(WARMUP_MMS):
                nc.tensor.matmul(wps, lhsT=warm[:, :128], rhs=warm,
                                 start=(w == 0), stop=(w == WARMUP_MMS - 1))
            nc.vector.tensor_copy(out=warm[:, 0:1], in_=wps[:, 0:1])

            states = [load_batch(b) for b in range(BPC)]
            work = [(b, qi) for b in range(BPC) for qi in range(NQT)]
            pending = None  # (segs, b, qi, p_sb)
            for b, qi in work:
                p_sb = stage1(states[b], qi)
                if pending is not None:
                    stage2(*pending)
                pending = (states[b], b, qi, p_sb)
            stage2(*pending)

    nc.compile()
    return nc


def _get_nc():
    if "nc" not in _NC_CACHE:
        _NC_CACHE["nc"] = _build_nc()
    return _NC_CACHE["nc"]


def _prepare(query, key, value, valid_length):
    query = np.asarray(query, dtype=np.float32)
    key = np.asarray(key, dtype=np.float32)
    value = np.asarray(value, dtype=np.float32)
    valid_length = np.asarray(valid_length)

    kz = key.copy()
    for b in range(B):
        kz[b, int(valid_length[b]):, :] = 0.0

    bf16 = ml_dtypes.bfloat16
    # ktc[c][b, p, k] = Kz[b, k, c*128+p]
    karr = kz.transpose(0, 2, 1).reshape(B, NDC, 128, LK)  # [B, c, p, k]
    # qc[b, p, c, q] = Q[b, q, c*128+p]
    qarr = query.transpose(0, 2, 1).reshape(B, NDC, 128, LQ) \
        .transpose(0, 2, 1, 3)  # [B, 128, NDC, LQ]
    qts = [qarr[:, :, :, qi * QT:(qi + 1) * QT].reshape(B, 128, NDC * QT)
           for qi in range(NQT)]
    # qt0 split per c-chunk for the A segments
    qt0c = [qarr[:, :, c, 0:QT] for c in range(NDC)]  # [B, 128, QT] each
    # v1h[b, p, t*VF + v] = [V|1][b, (8h+t)*128+p, v]
    v1 = np.concatenate(
        [value, np.ones((B, LK, 1), np.float32)], axis=-1)  # [B, LK, VF]
    v1arr = v1.reshape(B, NKT, 128, VF).transpose(0, 2, 1, 3)  # [B,128,NKT,VF]
    v1a = v1arr[:, :, 0:NKT // 2, :].reshape(B, 128, (NKT // 2) * VF)
    v1b = v1arr[:, :, NKT // 2:NKT, :].reshape(B, 128, (NKT // 2) * VF)

    blob = np.concatenate(
        [karr[:, 0], qt0c[0],          # seg A1
         karr[:, 1], qt0c[1],          # seg A2
         qts[1], v1a,                  # seg B
         qts[2], v1b,                  # seg C
         qts[3]],                      # seg D
        axis=2)
    assert blob.shape == (B, 128, BLOB)
    return np.ascontiguousarray(blob).astype(bf16)


def _run(inputs, trace=False):
    blob = _prepare(**inputs)
    in_maps = [{"blob": blob[c * BPC:(c + 1) * BPC]} for c in range(N_CORES)]
    nc = _get_nc()
    res = run_bass_kernel_spmd(nc, in_maps, core_ids=list(range(N_CORES)),
                               trace=trace)
    out = np.empty((B, LQ, DV), np.float32)
    for c in range(N_CORES):
        out[c * BPC:(c + 1) * BPC] = res.results[c]["out"]
    return out, res


def kernel(query, key, value, valid_length):
    out, _ = _run(dict(query=query, key=key, value=value,
                       valid_length=valid_length))
    return out


# revision 26
# speedup vs baseline: 1.0287x; 1.0287x over previous
"""Masked cross-attention kernel for Trainium2 (8 NeuronCores, SPMD).

Problem: B=16 batches of softmax(mask(Q@K^T/sqrt(D)))@V with
Lq=Lk=2048, D=DV=256.  The reference zeroes masked scores (NOT -inf)
before the softmax, so masked keys still contribute exp(0)=1 to the
denominator and weight 1/denom on V rows.

Strategy (all host prep is exact):
  * Zero K rows at k >= valid_length[b] on the host.  Then Q @ K^T is
    *exactly* 0.0 at masked positions - identical to the reference's
    jnp.where - and no mask tensor is needed on-device.
  * Pre-transpose Q and K to [D, L] layout on the host so both matmul
    operands stream naturally (contraction on the partition dim).
  * Append a ones-column to V.  P @ [V | 1] then yields the softmax
    denominator as output column 256 for free.
  * bf16 matmul inputs (fp32 PSUM accumulate), fp32 softmax math.
  * All per-batch inputs are packed host-side into ONE blob tensor
    [128 partitions x cols] and loaded in 3 big segment DMAs (a single
    DMA fans out across all 16 SDMA engines; many small DMAs each pay
    a ~2us completion latency and fair-share the engines).

Per core: 2 batches.  Per batch, for each 512-wide q tile:
  stage 1: S^T[k,q] tiles in PSUM (Kt.T @ Qt), exp via ScalarE
           (scale=1/16 folded in) -> P^T bf16 in SBUF
  stage 2: O[q,v] = (P^T).T @ [V|1] accumulated over k chunks in PSUM;
           divide by column 256 (DVE reciprocal + per-partition mul).
Stage 1 of q-tile i+1 is emitted before stage 2 of q-tile i so the PE
never stalls on the ScalarE exp chain.
"""

import numpy as np
import ml_dtypes

import concourse.bass as bass
import concourse.mybir as mybir
import concourse.tile as tile
from concourse import bacc
from concourse.bass_utils import run_bass_kernel_spmd

B, LQ, LK, D, DV = 16, 2048, 2048, 256, 256
N_CORES = 8
BPC = B // N_CORES  # batches per core

QT = 512            # q-tile width (stage-1 moving free dim)
NQT = LQ // QT      # 4
KT = 128            # k-tile (partition dim of S^T)
NKT = LK // KT      # 16
KG = 2              # k-tiles per PSUM/exp group
NKG = NKT // KG     # 8
NDC = D // 128      # contraction chunks (2)
QS = 128            # q-subtile for stage 2
NQS = QT // QS      # 4
VF = DV + 1         # 257: V plus the ones column
WARMUP_MMS = 10     # HAM warm-up matmuls bridging the initial DMA wait

# Blob column layout (per partition, bf16).  Segments sized so the
# latency-critical first working set (kt + qt0) splits evenly across
# the two independent HWDGE rings (sync + scalar), which run FIFO-serial
# per ring at ~170GB/s each:
#   seg A1: kt_c0 2048 | qt0_c0 512   -> 2560   (sync)
#   seg A2: kt_c1 2048 | qt0_c1 512   -> 2560   (scalar)
#   seg B:  qt1 1024   | v1a 8*VF=2056 -> 3080  (sync)
#   seg C:  qt2 1024   | v1b 2056      -> 3080  (scalar)
#   seg D:  qt3 1024                   -> 1024  (sync)
SEG_A = LK + QT                      # 2560 (x2)
SEG_B = NDC * QT + (NKT // 2) * VF   # 3080
SEG_C = SEG_B                        # 3080
SEG_D = NDC * QT                     # 1024
BLOB = 2 * SEG_A + SEG_B + SEG_C + SEG_D  # 12304

_BF16 = mybir.dt.bfloat16
_F32 = mybir.dt.float32

_NC_CACHE = {}


def _build_nc():
    nc = bacc.Bacc("TRN2", target_bir_lowering=False, debug=False,
                   num_devices=N_CORES)

    blob_d = nc.declare_dram_parameter("blob", [BPC, 128, BLOB], _BF16,
                                       isOutput=False)
    out_d = nc.declare_dram_parameter("out", [BPC, LQ, DV], _F32,
                                      isOutput=True)

    with tile.TileContext(nc) as tc:
        with (
            tc.tile_pool(name="seg", bufs=2) as seg_pool,
            tc.tile_pool(name="p", bufs=2) as p_pool,
            tc.tile_pool(name="osb", bufs=4) as o_pool,
            tc.tile_pool(name="small", bufs=8) as small_pool,
            tc.tile_pool(name="ps_s", bufs=2, space="PSUM") as ps_s,
            tc.tile_pool(name="ps_o", bufs=4, space="PSUM") as ps_o,
        ):
            SEG_TBL = [  # (offset, size)
                (0, SEG_A), (SEG_A, SEG_A), (2 * SEG_A, SEG_B),
                (2 * SEG_A + SEG_B, SEG_C), (2 * SEG_A + SEG_B + SEG_C, SEG_D),
            ]

            def load_batch(b):
                segs = []
                # batch 0 is latency-critical: A1/B/D on the sync ring,
                # A2/C on the scalar ring, concurrently.  batch 1 loads
                # all on sync (no deadline; keeps ACT free for exps).
                engs = ([nc.sync, nc.scalar, nc.sync, nc.scalar, nc.sync]
                        if b == 0 else [nc.sync] * 5)
                for si, (lo, n) in enumerate(SEG_TBL):
                    t = seg_pool.tile([128, n], _BF16, tag=f"seg{si}",
                                      name=f"seg{si}_b{b}")
                    engs[si].dma_start(out=t, in_=blob_d[b, :, lo:lo + n])
                    segs.append(t)
                return segs

            def kt_slice(segs, c, kj):
                return segs[c][:, kj * KT:(kj + 1) * KT]

            def qt_slice(segs, qi, c):
                if qi == 0:
                    return segs[c][:, LK:LK + QT]
                offs = [(2, 0), (3, 0), (4, 0)]
                si, o = offs[qi - 1]
                return segs[si][:, o + c * QT:o + (c + 1) * QT]

            def v1_slice(segs, kj):
                if kj < NKT // 2:
                    return segs[2][:, NDC * QT + kj * VF:NDC * QT + (kj + 1) * VF]
                kj -= NKT // 2
                return segs[3][:, NDC * QT + kj * VF:NDC * QT + (kj + 1) * VF]

            def stage1(segs, qi):
                """S^T = Kt.T @ Qt for one 512-wide q tile; exp -> P^T bf16."""
                p_sb = p_pool.tile([128, NKT * QT], _BF16, tag="p")
                for g in range(NKG):
                    ps = ps_s.tile([128, KG * QT], _F32, tag="s")
                    for h in range(KG):
                        kj = g * KG + h
                        for c in range(NDC):
                            nc.tensor.matmul(
                                ps[:, h * QT:(h + 1) * QT],
                                lhsT=kt_slice(segs, c, kj),
                                rhs=qt_slice(segs, qi, c),
                                start=(c == 0),
                                stop=(c == NDC - 1),
                            )
                    nc.scalar.activation(
                        out=p_sb[:, g * KG * QT:(g + 1) * KG * QT], in_=ps,
                        func=mybir.ActivationFunctionType.Exp,
                        scale=1.0 / 16.0)
                return p_sb

            def stage2(segs, b, qi, p_sb):
                """O = P @ [V|1]; normalize by the ones column; DMA out."""
                for s in range(NQS):
                    o_ps = ps_o.tile([128, VF], _F32, tag="o")
                    for kj in range(NKT):
                        nc.tensor.matmul(
                            o_ps,
                            lhsT=p_sb[:, kj * QT + s * QS:kj * QT + (s + 1) * QS],
                            rhs=v1_slice(segs, kj),
                            start=(kj == 0), stop=(kj == NKT - 1),
                        )
                    recip = small_pool.tile([128, 1], _F32, tag="r")
                    nc.vector.reciprocal(out=recip, in_=o_ps[:, DV:DV + 1])
                    o_sb = o_pool.tile([128, DV], _F32, tag="o_sb")
                    nc.vector.tensor_scalar_mul(
                        out=o_sb, in0=o_ps[:, :DV], scalar1=recip)
                    q0 = qi * QT + s * QS
                    nc.sync.dma_start(out=out_d[b, q0:q0 + QS, :], in_=o_sb)

            # HAM warm-up: matmuls on an UNINITIALIZED tile (no producer ->
            # no waits -> PE starts right after its preamble) into a
            # throwaway PSUM group.  The tiny DVE read keeps DCE away.
            warm = small_pool.tile([128, QT], _BF16, tag="warm")
            wps = ps_o.tile([128, QT], _F32, tag="o", name="warm_ps")
            for w in range(WARMUP_MMS):
                nc.tensor.matmul(wps, lhsT=warm[:, :128], rhs=warm,
                                 start=(w == 0), stop=(w == WARMUP_MMS - 1))
            nc.vector.tensor_copy(out=warm[:, 0:1], in_=wps[:, 0:1])

            states = [load_batch(b) for b in range(BPC)]
            work = [(b, qi) for b in range(BPC) for qi in range(NQT)]
            pending = None  # (segs, b, qi, p_sb)
            for b, qi in work:
                p_sb = stage1(states[b], qi)
                if pending is not None:
                    stage2(*pending)
                pending = (states[b], b, qi, p_sb)
            stage2(*pending)

    nc.compile()
    return nc


def _get_nc():
    if "nc" not in _NC_CACHE:
        _NC_CACHE["nc"] = _build_nc()
    return _NC_CACHE["nc"]


def _prepare(query, key, value, valid_length):
    query = np.asarray(query, dtype=np.float32)
    key = np.asarray(key, dtype=np.float32)
    value = np.asarray(value, dtype=np.float32)
    valid_length = np.asarray(valid_length)

    kz = key.copy()
    for b in range(B):
        kz[b, int(valid_length[b]):, :] = 0.0

    bf16 = ml_dtypes.bfloat16
    # ktc[c][b, p, k] = Kz[b, k, c*128+p]
    karr = kz.transpose(0, 2, 1).reshape(B, NDC, 128, LK)  # [B, c, p, k]
    # qc[b, p, c, q] = Q[b, q, c*128+p]
    qarr = query.transpose(0, 2, 1).reshape(B, NDC, 128, LQ) \
        .transpose(0, 2, 1, 3)  # [B, 128, NDC, LQ]
    qts = [qarr[:, :, :, qi * QT:(qi + 1) * QT].reshape(B, 128, NDC * QT)
           for qi in range(NQT)]
    # qt0 split per c-chunk for the A segments
    qt0c = [qarr[:, :, c, 0:QT] for c in range(NDC)]  # [B, 128, QT] each
    # v1h[b, p, t*VF + v] = [V|1][b, (8h+t)*128+p, v]
    v1 = np.concatenate(
        [value, np.ones((B, LK, 1), np.float32)], axis=-1)  # [B, LK, VF]
    v1arr = v1.reshape(B, NKT, 128, VF).transpose(0, 2, 1, 3)  # [B,128,NKT,VF]
    v1a = v1arr[:, :, 0:NKT // 2, :].reshape(B, 128, (NKT // 2) * VF)
    v1b = v1arr[:, :, NKT // 2:NKT, :].reshape(B, 128, (NKT // 2) * VF)

    blob = np.concatenate(
        [karr[:, 0], qt0c[0],          # seg A1
         karr[:, 1], qt0c[1],          # seg A2
         qts[1], v1a,                  # seg B
         qts[2], v1b,                  # seg C
         qts[3]],                      # seg D
        axis=2)
    assert blob.shape == (B, 128, BLOB)
    return np.ascontiguousarray(blob).astype(bf16)


def _run(inputs, trace=False):
    blob = _prepare(**inputs)
    in_maps = [{"blob": blob[c * BPC:(c + 1) * BPC]} for c in range(N_CORES)]
    nc = _get_nc()
    res = run_bass_kernel_spmd(nc, in_maps, core_ids=list(range(N_CORES)),
                               trace=trace)
    out = np.empty((B, LQ, DV), np.float32)
    for c in range(N_CORES):
        out[c * BPC:(c + 1) * BPC] = res.results[c]["out"]
    return out, res


def kernel(query, key, value, valid_length):
    out, _ = _run(dict(query=query, key=key, value=value,
                       valid_length=valid_length))
    return out


# revision 29
# speedup vs baseline: 1.0342x; 1.0054x over previous
"""Masked cross-attention kernel for Trainium2 (8 NeuronCores, SPMD).

Problem: B=16 batches of softmax(mask(Q@K^T/sqrt(D)))@V with
Lq=Lk=2048, D=DV=256.  The reference zeroes masked scores (NOT -inf)
before the softmax, so masked keys still contribute exp(0)=1 to the
denominator and weight 1/denom on V rows.

Strategy (all host prep is exact):
  * Zero K rows at k >= valid_length[b] on the host.  Then Q @ K^T is
    *exactly* 0.0 at masked positions - identical to the reference's
    jnp.where - and no mask tensor is needed on-device.
  * Pre-transpose Q and K to [D, L] layout on the host so both matmul
    operands stream naturally (contraction on the partition dim).
  * Append a ones-column to V.  P @ [V | 1] then yields the softmax
    denominator as output column 256 for free.
  * bf16 matmul inputs (fp32 PSUM accumulate), fp32 softmax math.
  * All per-batch inputs are packed host-side into ONE blob tensor
    [128 partitions x cols] and loaded in 3 big segment DMAs (a single
    DMA fans out across all 16 SDMA engines; many small DMAs each pay
    a ~2us completion latency and fair-share the engines).

Per core: 2 batches.  Per batch, for each 512-wide q tile:
  stage 1: S^T[k,q] tiles in PSUM (Kt.T @ Qt), exp via ScalarE
           (scale=1/16 folded in) -> P^T bf16 in SBUF
  stage 2: O[q,v] = (P^T).T @ [V|1] accumulated over k chunks in PSUM;
           divide by column 256 (DVE reciprocal + per-partition mul).
Stage 1 of q-tile i+1 is emitted before stage 2 of q-tile i so the PE
never stalls on the ScalarE exp chain.
"""

import numpy as np
import ml_dtypes

import concourse.bass as bass
import concourse.mybir as mybir
import concourse.tile as tile
from concourse import bacc
from concourse.bass_utils import run_bass_kernel_spmd

B, LQ, LK, D, DV = 16, 2048, 2048, 256, 256
N_CORES = 8
BPC = B // N_CORES  # batches per core

QT = 512            # q-tile width (stage-1 moving free dim)
NQT = LQ // QT      # 4
KT = 128            # k-tile (partition dim of S^T)
NKT = LK // KT      # 16
KG = 2              # k-tiles per PSUM/exp group
NKG = NKT // KG     # 8
NDC = D // 128      # contraction chunks (2)
QS = 128            # q-subtile for stage 2
NQS = QT // QS      # 4
VF = DV + 1         # 257: V plus the ones column
WARMUP_MMS = 10     # HAM warm-up matmuls bridging the initial DMA wait

# Blob column layout (per partition, bf16).  Segments sized so the
# latency-critical first working set (kt + qt0) splits evenly across
# the two independent HWDGE rings (sync + scalar), which run FIFO-serial
# per ring at ~170GB/s each:
#   seg A1: kt_c0 2048 | qt0_c0 512   -> 2560   (sync)
#   seg A2: kt_c1 2048 | qt0_c1 512   -> 2560   (scalar)
#   seg B:  qt1 1024   | v1a 8*VF=2056 -> 3080  (sync)
#   seg C:  qt2 1024   | v1b 2056      -> 3080  (scalar)
#   seg D:  qt3 1024                   -> 1024  (sync)
SEG_A = LK + QT                      # 2560 (x2)
SEG_B = NDC * QT + (NKT // 2) * VF   # 3080
SEG_C = SEG_B                        # 3080
SEG_D = NDC * QT                     # 1024
BLOB = 2 * SEG_A + SEG_B + SEG_C + SEG_D  # 12304

_BF16 = mybir.dt.bfloat16
_F32 = mybir.dt.float32

_NC_CACHE = {}


def _build_nc():
    nc = bacc.Bacc("TRN2", target_bir_lowering=False, debug=False,
                   num_devices=N_CORES)

    blob_d = nc.declare_dram_parameter("blob", [BPC, 128 * BLOB], _BF16,
                                       isOutput=False)
    out_d = nc.declare_dram_parameter("out", [BPC, LQ, DV], _F32,
                                      isOutput=True)

    with tile.TileContext(nc) as tc:
        with (
            tc.tile_pool(name="seg", bufs=2) as seg_pool,
            tc.tile_pool(name="p", bufs=2) as p_pool,
            tc.tile_pool(name="osb", bufs=4) as o_pool,
            tc.tile_pool(name="small", bufs=8) as small_pool,
            tc.tile_pool(name="ps_s", bufs=2, space="PSUM") as ps_s,
            tc.tile_pool(name="ps_o", bufs=4, space="PSUM") as ps_o,
        ):
            SEG_TBL = [  # (offset, size)
                (0, SEG_A), (SEG_A, SEG_A), (2 * SEG_A, SEG_B),
                (2 * SEG_A + SEG_B, SEG_C), (2 * SEG_A + SEG_B + SEG_C, SEG_D),
            ]

            def load_batch(b):
                segs = []
                # Each segment is a fully CONTIGUOUS block in DRAM
                # (partition-major within the segment), so the DMA is a
                # pure sequential HBM read - max fan-out and HBM
                # efficiency.  All on the sync ring, FIFO in deadline
                # order.
                for si, (lo, n) in enumerate(SEG_TBL):
                    t = seg_pool.tile([128, n], _BF16, tag=f"seg{si}",
                                      name=f"seg{si}_b{b}")
                    src = blob_d[b, 128 * lo:128 * (lo + n)].rearrange(
                        "(p n) -> p n", p=128)
                    nc.sync.dma_start(out=t, in_=src)
                    segs.append(t)
                return segs

            def kt_slice(segs, c, kj):
                return segs[c][:, kj * KT:(kj + 1) * KT]

            def qt_slice(segs, qi, c):
                if qi == 0:
                    return segs[c][:, LK:LK + QT]
                offs = [(2, 0), (3, 0), (4, 0)]
                si, o = offs[qi - 1]
                return segs[si][:, o + c * QT:o + (c + 1) * QT]

            def v1_slice(segs, kj):
                if kj < NKT // 2:
                    return segs[2][:, NDC * QT + kj * VF:NDC * QT + (kj + 1) * VF]
                kj -= NKT // 2
                return segs[3][:, NDC * QT + kj * VF:NDC * QT + (kj + 1) * VF]

            def stage1(segs, qi):
                """S^T = Kt.T @ Qt for one 512-wide q tile; exp -> P^T bf16."""
                p_sb = p_pool.tile([128, NKT * QT], _BF16, tag="p")
                for g in range(NKG):
                    ps = ps_s.tile([128, KG * QT], _F32, tag="s")
                    for h in range(KG):
                        kj = g * KG + h
                        for c in range(NDC):
                            nc.tensor.matmul(
                                ps[:, h * QT:(h + 1) * QT],
                                lhsT=kt_slice(segs, c, kj),
                                rhs=qt_slice(segs, qi, c),
                                start=(c == 0),
                                stop=(c == NDC - 1),
                            )
                    nc.scalar.activation(
                        out=p_sb[:, g * KG * QT:(g + 1) * KG * QT], in_=ps,
                        func=mybir.ActivationFunctionType.Exp,
                        scale=1.0 / 16.0)
                return p_sb

            def stage2(segs, b, qi, p_sb):
                """O = P @ [V|1]; normalize by the ones column; DMA out."""
                for s in range(NQS):
                    o_ps = ps_o.tile([128, VF], _F32, tag="o")
                    for kj in range(NKT):
                        nc.tensor.matmul(
                            o_ps,
                            lhsT=p_sb[:, kj * QT + s * QS:kj * QT + (s + 1) * QS],
                            rhs=v1_slice(segs, kj),
                            start=(kj == 0), stop=(kj == NKT - 1),
                        )
                    recip = small_pool.tile([128, 1], _F32, tag="r")
                    nc.vector.reciprocal(out=recip, in_=o_ps[:, DV:DV + 1])
                    o_sb = o_pool.tile([128, DV], _F32, tag="o_sb")
                    nc.vector.tensor_scalar_mul(
                        out=o_sb, in0=o_ps[:, :DV], scalar1=recip)
                    q0 = qi * QT + s * QS
                    nc.sync.dma_start(out=out_d[b, q0:q0 + QS, :], in_=o_sb)

            # HAM warm-up: matmuls on an UNINITIALIZED tile (no producer ->
            # no waits -> PE starts right after its preamble) into a
            # throwaway PSUM group.  The tiny DVE read keeps DCE away.
            warm = small_pool.tile([128, QT], _BF16, tag="warm")
            wps = ps_o.tile([128, QT], _F32, tag="o", name="warm_ps")
            for w in range(WARMUP_MMS):
                nc.tensor.matmul(wps, lhsT=warm[:, :128], rhs=warm,
                                 start=(w == 0), stop=(w == WARMUP_MMS - 1))
            nc.vector.tensor_copy(out=warm[:, 0:1], in_=wps[:, 0:1])

            states = [load_batch(b) for b in range(BPC)]
            work = [(b, qi) for b in range(BPC) for qi in range(NQT)]
            pending = None  # (segs, b, qi, p_sb)
            for b, qi in work:
                p_sb = stage1(states[b], qi)
                if pending is not None:
                    stage2(*pending)
                pending = (states[b], b, qi, p_sb)
            stage2(*pending)

    nc.compile()
    return nc


def _get_nc():
    if "nc" not in _NC_CACHE:
        _NC_CACHE["nc"] = _build_nc()
    return _NC_CACHE["nc"]


def _prepare(query, key, value, valid_length):
    query = np.asarray(query, dtype=np.float32)
    key = np.asarray(key, dtype=np.float32)
    value = np.asarray(value, dtype=np.float32)
    valid_length = np.asarray(valid_length)

    kz = key.copy()
    for b in range(B):
        kz[b, int(valid_length[b]):, :] = 0.0

    bf16 = ml_dtypes.bfloat16
    # ktc[c][b, p, k] = Kz[b, k, c*128+p]
    karr = kz.transpose(0, 2, 1).reshape(B, NDC, 128, LK)  # [B, c, p, k]
    # qc[b, p, c, q] = Q[b, q, c*128+p]
    qarr = query.transpose(0, 2, 1).reshape(B, NDC, 128, LQ) \
        .transpose(0, 2, 1, 3)  # [B, 128, NDC, LQ]
    qts = [qarr[:, :, :, qi * QT:(qi + 1) * QT].reshape(B, 128, NDC * QT)
           for qi in range(NQT)]
    # qt0 split per c-chunk for the A segments
    qt0c = [qarr[:, :, c, 0:QT] for c in range(NDC)]  # [B, 128, QT] each
    # v1h[b, p, t*VF + v] = [V|1][b, (8h+t)*128+p, v]
    v1 = np.concatenate(
        [value, np.ones((B, LK, 1), np.float32)], axis=-1)  # [B, LK, VF]
    v1arr = v1.reshape(B, NKT, 128, VF).transpose(0, 2, 1, 3)  # [B,128,NKT,VF]
    v1a = v1arr[:, :, 0:NKT // 2, :].reshape(B, 128, (NKT // 2) * VF)
    v1b = v1arr[:, :, NKT // 2:NKT, :].reshape(B, 128, (NKT // 2) * VF)

    seg_a1 = np.concatenate([karr[:, 0], qt0c[0]], axis=2)
    seg_a2 = np.concatenate([karr[:, 1], qt0c[1]], axis=2)
    seg_b = np.concatenate([qts[1], v1a], axis=2)
    seg_c = np.concatenate([qts[2], v1b], axis=2)
    seg_d = qts[3]
    # Flatten each segment partition-major so it is contiguous in DRAM.
    blob = np.concatenate(
        [s.reshape(B, -1) for s in (seg_a1, seg_a2, seg_b, seg_c, seg_d)],
        axis=1)
    assert blob.shape == (B, 128 * BLOB)
    return np.ascontiguousarray(blob).astype(bf16)


def _run(inputs, trace=False):
    blob = _prepare(**inputs)
    in_maps = [{"blob": blob[c * BPC:(c + 1) * BPC]} for c in range(N_CORES)]
    nc = _get_nc()
    res = run_bass_kernel_spmd(nc, in_maps, core_ids=list(range(N_CORES)),
                               trace=trace)
    out = np.empty((B, LQ, DV), np.float32)
    for c in range(N_CORES):
        out[c * BPC:(c + 1) * BPC] = res.results[c]["out"]
    return out, res


def kernel(query, key, value, valid_length):
    out, _ = _run(dict(query=query, key=key, value=value,
                       valid_length=valid_length))
    return out


# revision 33
# speedup vs baseline: 1.0584x; 1.0234x over previous
"""Masked cross-attention kernel for Trainium2 (8 NeuronCores, SPMD).

Problem: B=16 batches of softmax(mask(Q@K^T/sqrt(D)))@V with
Lq=Lk=2048, D=DV=256.  The reference zeroes masked scores (NOT -inf)
before the softmax, so masked keys still contribute exp(0)=1 to the
denominator and weight 1/denom on V rows.

Strategy (all host prep is exact):
  * Zero K rows at k >= valid_length[b] on the host.  Then Q @ K^T is
    *exactly* 0.0 at masked positions - identical to the reference's
    jnp.where - and no mask tensor is needed on-device.
  * Pre-transpose Q and K to [D, L] layout on the host so both matmul
    operands stream naturally (contraction on the partition dim).
  * Append a ones-column to V.  P @ [V | 1] then yields the softmax
    denominator as output column 256 for free.
  * bf16 matmul inputs (fp32 PSUM accumulate), fp32 softmax math.
  * All per-batch inputs are packed host-side into ONE blob tensor
    [128 partitions x cols] and loaded in 3 big segment DMAs (a single
    DMA fans out across all 16 SDMA engines; many small DMAs each pay
    a ~2us completion latency and fair-share the engines).

Per core: 2 batches.  Per batch, for each 512-wide q tile:
  stage 1: S^T[k,q] tiles in PSUM (Kt.T @ Qt), exp via ScalarE
           (scale=1/16 folded in) -> P^T bf16 in SBUF
  stage 2: O[q,v] = (P^T).T @ [V|1] accumulated over k chunks in PSUM;
           divide by column 256 (DVE reciprocal + per-partition mul).
Stage 1 of q-tile i+1 is emitted before stage 2 of q-tile i so the PE
never stalls on the ScalarE exp chain.
"""

import numpy as np
import ml_dtypes

import concourse.bass as bass
import concourse.mybir as mybir
import concourse.tile as tile
from concourse import bacc
from concourse.bass_utils import run_bass_kernel_spmd

B, LQ, LK, D, DV = 16, 2048, 2048, 256, 256
N_CORES = 8
BPC = B // N_CORES  # batches per core

QT = 512            # q-tile width (stage-1 moving free dim)
NQT = LQ // QT      # 4
KT = 128            # k-tile (partition dim of S^T)
NKT = LK // KT      # 16
KG = 2              # k-tiles per PSUM/exp group
NKG = NKT // KG     # 8
NDC = D // 128      # contraction chunks (2)
QS = 128            # q-subtile for stage 2
NQS = QT // QS      # 4
VF = DV + 1         # 257: V plus the ones column
WARMUP_MMS = 10     # HAM warm-up matmuls bridging the initial DMA wait

# Blob column layout (per partition, bf16).  Segments sized so the
# latency-critical first working set (kt + qt0) splits evenly across
# the two independent HWDGE rings (sync + scalar), which run FIFO-serial
# per ring at ~170GB/s each:
#   seg A1: kt[c0,k0:8] 1024 | kt[c1,k0:8] 1024 | qt0 1024 -> 3072
#           (everything stage-1 groups g0-g3 of q-tile 0 need)
#   seg A2: kt[c0,k8:16] 1024 | kt[c1,k8:16] 1024          -> 2048
#   seg B:  qt1 1024   | v1a 8*VF=2056 -> 3080
#   seg C:  qt2 1024   | v1b 2056      -> 3080
#   seg D:  qt3 1024                   -> 1024
HK = LK // 2                         # 1024 kt columns per c per segment
SEG_A1 = NDC * HK + NDC * QT         # 3072
SEG_A2 = NDC * HK                    # 2048
SEG_B = NDC * QT + (NKT // 2) * VF   # 3080
SEG_C = SEG_B                        # 3080
SEG_D = NDC * QT                     # 1024
BLOB = SEG_A1 + SEG_A2 + SEG_B + SEG_C + SEG_D  # 12304

_BF16 = mybir.dt.bfloat16
_F32 = mybir.dt.float32

_NC_CACHE = {}


def _build_nc():
    nc = bacc.Bacc("TRN2", target_bir_lowering=False, debug=False,
                   num_devices=N_CORES)

    blob_d = nc.declare_dram_parameter("blob", [BPC, 128 * BLOB], _BF16,
                                       isOutput=False)
    out_d = nc.declare_dram_parameter("out", [BPC, LQ, DV], _F32,
                                      isOutput=True)

    with tile.TileContext(nc) as tc:
        with (
            tc.tile_pool(name="seg", bufs=2) as seg_pool,
            tc.tile_pool(name="p", bufs=2) as p_pool,
            tc.tile_pool(name="osb", bufs=4) as o_pool,
            tc.tile_pool(name="small", bufs=8) as small_pool,
            tc.tile_pool(name="ps_s", bufs=2, space="PSUM") as ps_s,
            tc.tile_pool(name="ps_o", bufs=4, space="PSUM") as ps_o,
        ):
            SEG_SIZES = [SEG_A1, SEG_A2, SEG_B, SEG_C, SEG_D]
            SEG_TBL = []
            _off = 0
            for _n in SEG_SIZES:
                SEG_TBL.append((_off, _n))
                _off += _n

            def load_batch(b):
                segs = []
                # Each segment is a fully CONTIGUOUS block in DRAM
                # (partition-major within the segment), so the DMA is a
                # pure sequential HBM read - max fan-out and HBM
                # efficiency.  All on the sync ring, FIFO in deadline
                # order.
                for si, (lo, n) in enumerate(SEG_TBL):
                    t = seg_pool.tile([128, n], _BF16, tag=f"seg{si}",
                                      name=f"seg{si}_b{b}")
                    src = blob_d[b, 128 * lo:128 * (lo + n)].rearrange(
                        "(p n) -> p n", p=128)
                    nc.sync.dma_start(out=t, in_=src)
                    segs.append(t)
                return segs

            def kt_slice(segs, c, kj):
                h, ko = kj // (NKT // 2), kj % (NKT // 2)
                o = c * HK + ko * KT
                return segs[h][:, o:o + KT]

            def qt_slice(segs, qi, c):
                if qi == 0:
                    return segs[0][:, NDC * HK + c * QT:NDC * HK + (c + 1) * QT]
                offs = [(2, 0), (3, 0), (4, 0)]
                si, o = offs[qi - 1]
                return segs[si][:, o + c * QT:o + (c + 1) * QT]

            def v1_slice(segs, kj):
                if kj < NKT // 2:
                    return segs[2][:, NDC * QT + kj * VF:NDC * QT + (kj + 1) * VF]
                kj -= NKT // 2
                return segs[3][:, NDC * QT + kj * VF:NDC * QT + (kj + 1) * VF]

            def stage1(segs, qi):
                """S^T = Kt.T @ Qt for one 512-wide q tile; exp -> P^T bf16."""
                p_sb = p_pool.tile([128, NKT * QT], _BF16, tag="p")
                for g in range(NKG):
                    ps = ps_s.tile([128, KG * QT], _F32, tag="s")
                    for h in range(KG):
                        kj = g * KG + h
                        for c in range(NDC):
                            nc.tensor.matmul(
                                ps[:, h * QT:(h + 1) * QT],
                                lhsT=kt_slice(segs, c, kj),
                                rhs=qt_slice(segs, qi, c),
                                start=(c == 0),
                                stop=(c == NDC - 1),
                            )
                    nc.scalar.activation(
                        out=p_sb[:, g * KG * QT:(g + 1) * KG * QT], in_=ps,
                        func=mybir.ActivationFunctionType.Exp,
                        scale=1.0 / 16.0)
                return p_sb

            def stage2(segs, b, qi, p_sb):
                """O = P @ [V|1]; normalize by the ones column; DMA out."""
                for s in range(NQS):
                    o_ps = ps_o.tile([128, VF], _F32, tag="o")
                    for kj in range(NKT):
                        nc.tensor.matmul(
                            o_ps,
                            lhsT=p_sb[:, kj * QT + s * QS:kj * QT + (s + 1) * QS],
                            rhs=v1_slice(segs, kj),
                            start=(kj == 0), stop=(kj == NKT - 1),
                        )
                    recip = small_pool.tile([128, 1], _F32, tag="r")
                    nc.vector.reciprocal(out=recip, in_=o_ps[:, DV:DV + 1])
                    o_sb = o_pool.tile([128, DV], _F32, tag="o_sb")
                    nc.vector.tensor_scalar_mul(
                        out=o_sb, in0=o_ps[:, :DV], scalar1=recip)
                    q0 = qi * QT + s * QS
                    nc.sync.dma_start(out=out_d[b, q0:q0 + QS, :], in_=o_sb)

            # HAM warm-up: matmuls on an UNINITIALIZED tile (no producer ->
            # no waits -> PE starts right after its preamble) into a
            # throwaway PSUM group.  The tiny DVE read keeps DCE away.
            warm = small_pool.tile([128, QT], _BF16, tag="warm")
            wps = ps_o.tile([128, QT], _F32, tag="o", name="warm_ps")
            for w in range(WARMUP_MMS):
                nc.tensor.matmul(wps, lhsT=warm[:, :128], rhs=warm,
                                 start=(w == 0), stop=(w == WARMUP_MMS - 1))
            nc.vector.tensor_copy(out=warm[:, 0:1], in_=wps[:, 0:1])

            states = [load_batch(b) for b in range(BPC)]
            work = [(b, qi) for b in range(BPC) for qi in range(NQT)]
            pending = None  # (segs, b, qi, p_sb)
            for b, qi in work:
                p_sb = stage1(states[b], qi)
                if pending is not None:
                    stage2(*pending)
                pending = (states[b], b, qi, p_sb)
            stage2(*pending)

    nc.compile()
    return nc


def _get_nc():
    if "nc" not in _NC_CACHE:
        _NC_CACHE["nc"] = _build_nc()
    return _NC_CACHE["nc"]


def _prepare(query, key, value, valid_length):
    query = np.asarray(query, dtype=np.float32)
    key = np.asarray(key, dtype=np.float32)
    value = np.asarray(value, dtype=np.float32)
    valid_length = np.asarray(valid_length)

    kz = key.copy()
    for b in range(B):
        kz[b, int(valid_length[b]):, :] = 0.0

    bf16 = ml_dtypes.bfloat16
    # ktc[c][b, p, k] = Kz[b, k, c*128+p]
    karr = kz.transpose(0, 2, 1).reshape(B, NDC, 128, LK)  # [B, c, p, k]
    # qc[b, p, c, q] = Q[b, q, c*128+p]
    qarr = query.transpose(0, 2, 1).reshape(B, NDC, 128, LQ) \
        .transpose(0, 2, 1, 3)  # [B, 128, NDC, LQ]
    qts = [qarr[:, :, :, qi * QT:(qi + 1) * QT].reshape(B, 128, NDC * QT)
           for qi in range(NQT)]
    # qt0 split per c-chunk for the A segments
    qt0c = [qarr[:, :, c, 0:QT] for c in range(NDC)]  # [B, 128, QT] each
    # v1h[b, p, t*VF + v] = [V|1][b, (8h+t)*128+p, v]
    v1 = np.concatenate(
        [value, np.ones((B, LK, 1), np.float32)], axis=-1)  # [B, LK, VF]
    v1arr = v1.reshape(B, NKT, 128, VF).transpose(0, 2, 1, 3)  # [B,128,NKT,VF]
    v1a = v1arr[:, :, 0:NKT // 2, :].reshape(B, 128, (NKT // 2) * VF)
    v1b = v1arr[:, :, NKT // 2:NKT, :].reshape(B, 128, (NKT // 2) * VF)

    seg_a1 = np.concatenate(
        [karr[:, 0, :, 0:HK], karr[:, 1, :, 0:HK], qt0c[0], qt0c[1]], axis=2)
    seg_a2 = np.concatenate(
        [karr[:, 0, :, HK:LK], karr[:, 1, :, HK:LK]], axis=2)
    seg_b = np.concatenate([qts[1], v1a], axis=2)
    seg_c = np.concatenate([qts[2], v1b], axis=2)
    seg_d = qts[3]
    # Flatten each segment partition-major so it is contiguous in DRAM.
    blob = np.concatenate(
        [s.reshape(B, -1) for s in (seg_a1, seg_a2, seg_b, seg_c, seg_d)],
        axis=1)
    assert blob.shape == (B, 128 * BLOB)
    return np.ascontiguousarray(blob).astype(bf16)


def _run(inputs, trace=False):
    blob = _prepare(**inputs)
    in_maps = [{"blob": blob[c * BPC:(c + 1) * BPC]} for c in range(N_CORES)]
    nc = _get_nc()
    res = run_bass_kernel_spmd(nc, in_maps, core_ids=list(range(N_CORES)),
                               trace=trace)
    out = np.empty((B, LQ, DV), np.float32)
    for c in range(N_CORES):
        out[c * BPC:(c + 1) * BPC] = res.results[c]["out"]
    return out, res


def kernel(query, key, value, valid_length):
    out, _ = _run(dict(query=query, key=key, value=value,
                       valid_length=valid_length))
    return out
